# revision 1
# baseline (speedup 1.0000x reference)
"""Swin-style windowed attention TRN2 kernel (v2).

Math per window (n=49 tokens, d=128, 4 heads x 32):
  qkv = x @ W_qkv ; q *= dh**-0.5
  sim[h] = q_h @ k_h^T + bias[h] ; attn = softmax_j(sim)
  out = (attn @ v) @ W_out

v2 layout (per QUAD = 4 windows = 2 pairs):
  - x pair [98,128] --PE transpose--> xT cols of [128, 196] (bf16)
  - qT/kT head-split [32, 4, 196] (sim operands need base partition 0)
  - sim psum [128, 392]: two pair-stacks side by side; within a stack,
    window A rows 0:49, window B rows 64:113 (no cross-window garbage)
  - per (stack, wblock): bias-init matmul at matching tile_position, then
    per-head sim matmuls accumulate (consistent position per group)
  - ONE exp per quad [128, 392] -> U bf16 ; DVE reduce -> s [128, 8]
  - reciprocal ; ONE normalize op with step-0 broadcast of r
  - 16 PE transposes -> UT [49, 16, 49] ; AV -> av [64, 2, 4, 49]
  - proj per pair: 4 accumulating K=32 matmuls -> [98, 128] -> DMA out
"""

import os
import numpy as np
import ml_dtypes

import concourse.bass as bass
import concourse.mybir as mybir
import concourse.tile as tile
from concourse import bacc
from concourse.bass_utils import run_bass_kernel_spmd

DIM = 128
DH = 32
HEADS = 4
WS = 7
N = 49
SCALE = DH ** -0.5
P = 98            # tokens per window pair
QW = 4            # windows per quad
QT = 196          # tokens per quad
NCORES = 8
GROUP = 8         # pairs per DMA group (= 4 quads)

F32 = mybir.dt.float32
BF16 = mybir.dt.bfloat16
BF = ml_dtypes.bfloat16


def _rel_pos_bias(bias_table):
    pos = np.arange(WS)
    gi, gj = np.meshgrid(pos, pos, indexing="ij")
    grid = np.stack([gi, gj], -1).reshape(N, 2)
    rel = grid[:, None, :] - grid[None, :, :] + (WS - 1)
    idx = rel[..., 0] * (2 * WS - 1) + rel[..., 1]          # [N, N] int
    b = np.asarray(bias_table, np.float32)[idx]             # [N, N, H]
    return np.transpose(b, (2, 0, 1))                       # [H, N, N]


def _build_bias_block(bias_table):
    """[64, 4*49]: rows i (49 real + 15 pad), cols (h, j)."""
    bh = _rel_pos_bias(bias_table)
    out = np.zeros((64, HEADS * N), np.float32)
    for h in range(HEADS):
        out[:N, N * h:N * h + N] = bh[h]
    return out


def build_program(n_pairs, group=GROUP, repeats=1):
    nc = bacc.Bacc("TRN2", target_bir_lowering=False)
    TOK = n_pairs * P
    n_groups = n_pairs // group
    assert n_pairs % group == 0 and group % 2 == 0
    quads_per_group = group // 2
    SKIP = os.environ.get("KSKIP", "").split(",")

    x_d = nc.declare_dram_parameter("x", [TOK, DIM], F32, isOutput=False)
    wq_d = nc.declare_dram_parameter("wq", [DIM, DIM], BF16, isOutput=False)
    wk_d = nc.declare_dram_parameter("wk", [DIM, DIM], BF16, isOutput=False)
    wv_d = nc.declare_dram_parameter("wv", [DIM, DIM], BF16, isOutput=False)
    wo_d = nc.declare_dram_parameter("wo", [64, 2, DIM], BF16, isOutput=False)
    bias_d = nc.declare_dram_parameter("biasb", [64, HEADS * N], BF16, isOutput=False)
    i98_d = nc.declare_dram_parameter("i98", [DIM, DIM], BF16, isOutput=False)
    out_d = nc.declare_dram_parameter("out", [TOK, DIM], F32, isOutput=True)

    with tile.TileContext(nc) as tc:
        with (
            tc.tile_pool(name="const", bufs=1) as constp,
            tc.tile_pool(name="stage", bufs=4) as stagep,
            tc.tile_pool(name="xt", bufs=4) as xtp,
            tc.tile_pool(name="qk", bufs=4) as qkp,
            tc.tile_pool(name="vn", bufs=4) as vnp,
            tc.tile_pool(name="u", bufs=5) as up,
            tc.tile_pool(name="sr", bufs=6) as srp,
            tc.tile_pool(name="ut", bufs=4) as utp_pool,
            tc.tile_pool(name="ot", bufs=4) as otp,
            tc.tile_pool(name="fin", bufs=4) as finp,
            tc.tile_pool(name="psCD", bufs=int(os.environ.get("BCD", "3")),
                         space="PSUM") as psCD,
            tc.tile_pool(name="psWK", bufs=int(os.environ.get("BWK", "5")),
                         space="PSUM") as psWK,
        ):
            wq = constp.tile([DIM, DIM], BF16)
            nc.sync.dma_start(out=wq[:], in_=wq_d[:])
            wk = constp.tile([DIM, DIM], BF16)
            nc.sync.dma_start(out=wk[:], in_=wk_d[:])
            wv = constp.tile([DIM, DIM], BF16)
            nc.sync.dma_start(out=wv[:], in_=wv_d[:])
            wo2 = constp.tile([64, 2, DIM], BF16)
            nc.sync.dma_start(out=wo2[:], in_=wo_d[:])
            biasb = constp.tile([64, HEADS * N], BF16)
            nc.sync.dma_start(out=biasb[:], in_=bias_d[:])
            i98 = constp.tile([DIM, DIM], BF16)
            nc.sync.dma_start(out=i98[:], in_=i98_d[:])

            for _rep, g in [(rr, gg) for rr in range(repeats)
                            for gg in range(n_groups)]:
                r0 = g * group * P
                xs = stagep.tile([P, group, DIM], BF16, tag="xs")
                nc.gpsimd.dma_start(
                    out=xs[:],
                    in_=x_d[r0:r0 + group * P, :].rearrange(
                        "(p t) d -> t p d", p=group),
                )
                fs = finp.tile([P, group, DIM], F32, tag="fs")
                nc.scalar.memzero(fs[0:1, 0:1, 0:1])
                for q in range(quads_per_group):
                    # ---------- x transpose (2 pairs) ----------
                    pcd = psCD.tile([DIM, 490], F32, tag="psCD")
                    simp = pcd[:, 0:2 * HEADS * N]          # [128, 392] f32
                    xt_ps = pcd[:, 392:490].bitcast(BF16)   # [128, 196] bf16
                    for s in range(2):
                        nc.tensor.transpose(
                            xt_ps[:, P * s:P * s + P],
                            xs[:, 2 * q + s, :], i98[0:P, 0:P])
                    xt = xtp.tile([DIM, QT], BF16, tag="xt")
                    nc.vector.tensor_copy(xt[:], xt_ps)
                    # ---------- qkv into one packed work bank ----------
                    # rows 0:32 q | 32:64 k | 64:113 v ; later reused for
                    # put (rows 0:49), av (rows 0:32), f (rows 0:98)
                    wk_t = psWK.tile([DIM, 512], F32, tag="pswk")
                    qt = qkp.tile([DH, HEADS, QT], BF16, tag="qt")
                    kt = qkp.tile([DH, HEADS, QT], BF16, tag="kt")
                    for r in range(2):
                        pq = wk_t[0:DH, 0:HEADS * P].rearrange(
                            "p (h t) -> p h t", h=HEADS)
                        pk = wk_t[DH:2 * DH, 0:HEADS * P].rearrange(
                            "p (h t) -> p h t", h=HEADS)
                        for h in range(HEADS):
                            nc.tensor.matmul(pq[:, h, :],
                                             lhsT=wq[:, DH * h:DH * h + DH],
                                             rhs=xt[:, P * r:P * r + P])
                            nc.tensor.matmul(pk[:, h, :],
                                             lhsT=wk[:, DH * h:DH * h + DH],
                                             rhs=xt[:, P * r:P * r + P])
                        if "qkc" not in SKIP:
                            nc.vector.tensor_copy(qt[:, :, P * r:P * r + P], pq[:])
                            nc.scalar.copy(kt[:, :, P * r:P * r + P], pk[:])
                        else:
                            nc.scalar.copy(qt[:, 0:1, P * r:P * r + 2],
                                           pq[:, 0:1, 0:2])
                            nc.scalar.copy(kt[:, 0:1, P * r:P * r + 2],
                                           pk[:, 0:1, 0:2])
                    # ---------- v natural ----------
                    vn = vnp.tile([N, QW, DIM], BF16, tag="vn")
                    pv = wk_t[64:64 + N, :].rearrange("p (w d) -> p w d", w=QW)
                    for w in range(QW):
                        nc.tensor.matmul(
                            pv[:, w, :],
                            lhsT=xt[:, N * w:N * w + N],
                            rhs=wv[:])
                    nc.scalar.copy(vn[:], pv[:])
                    STAGE = int(os.environ.get("KSTAGE", "9"))
                    if STAGE < 3:
                        nc.scalar.copy(fs[:, 2 * q, :], pvf[0][0:P, 0:128])
                        nc.scalar.copy(fs[:, 2 * q + 1, :], pvf[1][0:P, 0:128])
                        continue
                    # ---------- sim: bias init + head matmuls ----------
                    for s in range(2):
                        for w in range(2):
                            nc.tensor.matmul(
                                simp[64 * w:64 * w + 64,
                                     HEADS * N * s:HEADS * N * (s + 1)],
                                lhsT=i98[0:64, 0:64], rhs=biasb[:],
                                start=True, stop=False,
                                skip_group_check=True)
                            for h in range(HEADS):
                                c0 = HEADS * N * s + N * h
                                t0 = P * s + N * w
                                nc.tensor.matmul(
                                    simp[64 * w:64 * w + N, c0:c0 + N],
                                    lhsT=qt[:, h, t0:t0 + N],
                                    rhs=kt[:, h, t0:t0 + N],
                                    start=False, stop=True,
                                    skip_group_check=True)
                    if STAGE < 4:
                        nc.scalar.copy(fs[:, 2 * q, :], simp[0:P, 0:128])
                        nc.scalar.copy(fs[:, 2 * q + 1, :], simp[0:P, 128:256])
                        continue
                    # ---------- softmax ----------
                    u = up.tile([DIM, 2 * HEADS * N], BF16, tag="u")
                    if "exp" not in SKIP:
                        nc.scalar.activation(u[:], simp,
                                             func=mybir.ActivationFunctionType.Exp)
                    else:
                        nc.scalar.memzero(u[:, 0:2])
                    sm = srp.tile([DIM, 2 * HEADS], F32, tag="s")
                    if "red" not in SKIP:
                        nc.vector.tensor_reduce(
                            sm[:], u[:].rearrange("p (a j) -> p a j", j=N),
                            axis=mybir.AxisListType.X, op=mybir.AluOpType.add)
                    else:
                        nc.vector.memset(sm[:], 1.0)
                    r_ = srp.tile([DIM, 2 * HEADS], F32, tag="r")
                    if "red" not in SKIP:
                        nc.vector.reciprocal(r_[:], sm[:])
                    else:
                        nc.vector.memset(r_[:], 1.0)
                    u2 = up.tile([DIM, 2 * HEADS * N], BF16, tag="u2")
                    r_b = bass.AP(
                        tensor=r_[:].tensor, offset=r_[:].offset,
                        ap=[list(r_[:].ap[0]), list(r_[:].ap[1]), [0, N]])
                    if "norm" not in SKIP:
                        nc.vector.tensor_mul(
                            u2[:].rearrange("p (a j) -> p a j", j=N),
                            u[:].rearrange("p (a j) -> p a j", j=N), r_b)
                    else:
                        nc.vector.memset(u2[:, 0:2], 1.0)
                    if STAGE < 5:
                        nc.scalar.copy(fs[:, 2 * q, :], simp[0:P, 0:128])
                        nc.vector.tensor_copy(fs[:, 2 * q + 1, 0:8], r_[0:P, :])
                        continue
                    # ---------- transpose attn ----------
                    # one transpose per (stack, head): [113, 49] -> [49, 113]
                    # cols 0:49 = window A's UT, 64:113 = window B's UT
                    put = wk_t[0:N, 0:464].bitcast(BF16).rearrange(
                        "p (b c) -> p b c", b=2 * HEADS)
                    for s in range(2):
                        for h in range(HEADS):
                            b = HEADS * s + h
                            nc.tensor.transpose(
                                put[:, b, 0:113],
                                u2[0:113,
                                   HEADS * N * s + N * h:
                                   HEADS * N * s + N * h + N],
                                i98[0:113, 0:113])
                    ut = utp_pool.tile([N, 2 * HEADS, 116], BF16, tag="ut")
                    if "utc" not in SKIP:
                        nc.vector.tensor_copy(ut[:], put[:])
                    else:
                        nc.vector.memset(ut[:, 0:1, 0:2], 1.0)
                    if STAGE < 6:
                        nc.vector.tensor_copy(fs[0:N, 2 * q, 0:49], ut[:, 0, :])
                        nc.scalar.copy(fs[:, 2 * q + 1, :], simp[0:P, 0:128])
                        continue
                    # ---------- attn @ v (head-pairs stacked, base 0/32) ----------
                    pav = wk_t[0:64, 0:392].rearrange(
                        "p (hh w c) -> p hh w c", hh=2, w=QW)
                    for s in range(2):
                        for w in range(2):
                            wi = 2 * s + w
                            for h in range(HEADS):
                                nc.tensor.matmul(
                                    pav[DH * (h % 2):DH * (h % 2) + DH,
                                        h // 2, wi, :],
                                    lhsT=vn[:, wi, DH * h:DH * h + DH],
                                    rhs=ut[:, HEADS * s + h,
                                           64 * w:64 * w + N])
                    ot = otp.tile([64, 2, QW, N], BF16, tag="ot")
                    if "otc" not in SKIP:
                        nc.vector.tensor_copy(ot[:], pav[:])
                    else:
                        nc.vector.memset(ot[:, 0:1, 0:1, 0:2], 1.0)
                    if STAGE < 7:
                        nc.scalar.copy(fs[0:P, 2 * q, 64:128], simp[0:P, 0:64])
                        nc.scalar.copy(fs[:, 2 * q + 1, :], simp[0:P, 0:128])
                        continue
                    # ---------- projection per pair (K=64 head-pairs) ----------
                    for s in range(2):
                        f_ps = wk_t[0:P, 256 + 128 * s:384 + 128 * s]
                        for hh in range(2):
                            nc.tensor.matmul(
                                f_ps,
                                lhsT=ot[:, hh, 2 * s:2 * s + 2, :],
                                rhs=wo2[:, hh, :],
                                start=(hh == 0), stop=(hh == 1))
                    nc.scalar.copy(
                        fs[:, 2 * q:2 * q + 2, :],
                        wk_t[0:P, 256:512].rearrange("p (s d) -> p s d", s=2))
                nc.sync.dma_start(
                    out=out_d[r0:r0 + group * P, :].rearrange(
                        "(p t) d -> t p d", p=group),
                    in_=fs[:],
                )
    nc.finalize()
    return nc


_CACHE = {}


def _get_program(n_pairs):
    if n_pairs not in _CACHE:
        _CACHE[n_pairs] = build_program(n_pairs)
    return _CACHE[n_pairs]


def _host_inputs(W_qkv, W_out, bias_table):
    W_qkv = np.asarray(W_qkv, np.float32)
    wo = np.asarray(W_out, np.float32).reshape(2, 64, DIM).transpose(1, 0, 2)
    return {
        "wq": np.ascontiguousarray((W_qkv[:, :DIM] * SCALE)).astype(BF),
        "wk": np.ascontiguousarray(W_qkv[:, DIM:2 * DIM]).astype(BF),
        "wv": np.ascontiguousarray(W_qkv[:, 2 * DIM:]).astype(BF),
        "wo": np.ascontiguousarray(wo).astype(BF),
        "biasb": _build_bias_block(bias_table).astype(BF),
        "i98": np.eye(DIM, dtype=np.float32).astype(BF),
    }


def kernel(x, W_qkv, W_out, bias_table):
    x = np.asarray(x, np.float32)
    shp = x.shape
    xf = np.ascontiguousarray(x.reshape(-1, DIM))
    tok = xf.shape[0]
    per = tok // NCORES
    n_pairs = per // P
    assert per % P == 0
    nc = _get_program(n_pairs)
    consts = _host_inputs(W_qkv, W_out, bias_table)
    in_maps = []
    for c in range(NCORES):
        m = {"x": np.ascontiguousarray(xf[c * per:(c + 1) * per])}
        m.update(consts)
        in_maps.append(m)
    res = run_bass_kernel_spmd(nc, in_maps, list(range(NCORES)))
    outs = [res.results[c]["out"] for c in range(NCORES)]
    return np.concatenate(outs, 0).reshape(shp).astype(np.float32)



# revision 39
# speedup vs baseline: 1.1517x; 1.1517x over previous
"""Swin-style windowed attention TRN2 kernel (v3.2).

Math per window (n=49 tokens, d=128, 4 heads x 32):
  qkv = x @ W_qkv ; q *= dh**-0.5
  sim[h] = q_h @ k_h^T + bias[h] ; attn = softmax_j(sim)
  out = (attn @ v) @ W_out

Deep software pipeline over QUADs (4 windows = 2 pairs = 196 tokens).
All matmul operands sit at partition base 0 (tile_position row 0 —
switching the row quadrant in-flight corrupts results on HW); only
column positions {0, 32, 64} are used.  Relative-position bias is
MULTIPLICATIVE: u = exp(sim) * exp(bias), with exp(bias) a precomputed
constant, so every sim matmul is a standalone start/stop group.
Head-split of the full-width q/k projection is done by 4 small
SBUF->SBUF DMAs on otherwise idle DMA queues.

Iteration t phases (quad index):
  qX=t     xT x2 -> W(t) cols 392:490 (bf16); xt copy
  qA=t-2   q,k M=128 x2 + v x4 -> W(t)/V(t); qk copy; qkh split DMAs;
           vn copy
  qB=t-4   sim x16 (K=32, M=64, col positions 64w) -> S(t)
  qE=t-5   exp ACT; expb mul (Pool); reduce+recip DVE; norm mul (Pool)
  qT=t-7   uT x8 per-head transposes [113,64]->[64,113] -> CU
  qV=t-8   av x16 (v2-style: out rows 32*(h%2), col-groups h//2)
  qP=t-10  proj x2x2 K=64 accum -> S(t-6) cols 0:256; fs copy;
           group out DMA
PSUM: W2 + V2 + S2 + CU2 = 8 banks, double-buffered.
"""

import os
import numpy as np
import ml_dtypes

import concourse.bass as bass
import concourse.mybir as mybir
import concourse.tile as tile
from concourse import bacc
from concourse.bass_utils import run_bass_kernel_spmd

DIM = 128
DH = 32
HEADS = 4
WS = 7
N = 49
SCALE = DH ** -0.5
P = 98            # tokens per window pair
NCORES = 8
GROUP = 8         # pairs per input DMA group (= 4 quads)

F32 = mybir.dt.float32
BF16 = mybir.dt.bfloat16
BF = ml_dtypes.bfloat16
Exp = mybir.ActivationFunctionType.Exp


def _rel_pos_bias(bias_table):
    pos = np.arange(WS)
    gi, gj = np.meshgrid(pos, pos, indexing="ij")
    grid = np.stack([gi, gj], -1).reshape(N, 2)
    rel = grid[:, None, :] - grid[None, :, :] + (WS - 1)
    idx = rel[..., 0] * (2 * WS - 1) + rel[..., 1]          # [N, N] int
    b = np.asarray(bias_table, np.float32)[idx]             # [N, N, H]
    return np.transpose(b, (2, 0, 1))                       # [H, N, N]


def _build_expb(bias_table):
    """[128, 4*64]: exp(bias), rows i repeated at 64 offset, cols (h, jp)."""
    bh = _rel_pos_bias(bias_table)
    out = np.ones((DIM, HEADS * 64), np.float32)
    for h in range(HEADS):
        blk = np.exp(bh[h])                                  # [49, 49]
        out[:N, 64 * h:64 * h + N] = blk
        out[64:64 + N, 64 * h:64 * h + N] = blk
    return out


def build_program(n_pairs, group=GROUP, repeats=1):
    nc = bacc.Bacc("TRN2", target_bir_lowering=False)
    TOK = n_pairs * P
    n_groups = n_pairs // group
    assert n_pairs % group == 0 and group == 8
    NQ = n_pairs // 2          # quads per repeat
    TOT = NQ * repeats
    TOTG = n_groups * repeats

    x_d = nc.declare_dram_parameter("x", [TOK, DIM], F32, isOutput=False)
    wq_d = nc.declare_dram_parameter("wq", [DIM, DIM], BF16, isOutput=False)
    wk_d = nc.declare_dram_parameter("wk", [DIM, DIM], BF16, isOutput=False)
    wv_d = nc.declare_dram_parameter("wv", [DIM, DIM], BF16, isOutput=False)
    wo_d = nc.declare_dram_parameter("wo", [64, 2, DIM], BF16, isOutput=False)
    expb_d = nc.declare_dram_parameter("expb", [DIM, HEADS * 64], BF16,
                                       isOutput=False)
    i128_d = nc.declare_dram_parameter("i128", [DIM, DIM], BF16, isOutput=False)
    out_d = nc.declare_dram_parameter("out", [TOK, DIM], F32, isOutput=True)

    with tile.TileContext(nc) as tc:
        with (
            tc.tile_pool(name="const", bufs=1) as constp,
            tc.tile_pool(name="stage", bufs=2) as stagep,
            tc.tile_pool(name="xts", bufs=4) as xtsp,
            tc.tile_pool(name="qks", bufs=3) as qksp,
            tc.tile_pool(name="qkh", bufs=3) as qkhp,
            tc.tile_pool(name="vns", bufs=8) as vnsp,
            tc.tile_pool(name="us", bufs=3) as usp,
            tc.tile_pool(name="u2s", bufs=4) as u2sp,
            tc.tile_pool(name="sr", bufs=3) as srp,
            tc.tile_pool(name="uts", bufs=3) as utsp,
            tc.tile_pool(name="ots", bufs=4) as otsp,
            tc.tile_pool(name="fin", bufs=3) as finp,
            tc.tile_pool(name="psW", bufs=2, space="PSUM") as psW,
            tc.tile_pool(name="psV", bufs=2, space="PSUM") as psV,
            tc.tile_pool(name="psS", bufs=2, space="PSUM") as psS,
            tc.tile_pool(name="psCU", bufs=2, space="PSUM") as psCU,
        ):
            wqS = constp.tile([DIM, DIM], BF16)
            nc.sync.dma_start(out=wqS[:], in_=wq_d[:])
            wkS = constp.tile([DIM, DIM], BF16)
            nc.sync.dma_start(out=wkS[:], in_=wk_d[:])
            wvS = constp.tile([DIM, DIM], BF16)
            nc.sync.dma_start(out=wvS[:], in_=wv_d[:])
            wo2 = constp.tile([64, 2, DIM], BF16)
            nc.sync.dma_start(out=wo2[:], in_=wo_d[:])
            expb = constp.tile([DIM, HEADS * 64], BF16)
            nc.sync.dma_start(out=expb[:], in_=expb_d[:])
            i128 = constp.tile([DIM, DIM], BF16)
            nc.sync.dma_start(out=i128[:], in_=i128_d[:])

            # pre-zero j-pad columns of all u2 slots (keeps transposed
            # garbage rows finite; av never reads them)
            for _ in range(4):
                u2z = u2sp.tile([DIM, 2 * HEADS, 64], BF16, tag="u2")
                nc.vector.memset(u2z[:, :, N:64], 0.0)

            xs_h = {}     # group -> staging tile
            fs_h = {}     # group -> out staging tile
            W_h = {}      # iter -> W psum tile
            CU_h = {}     # quad -> CU psum tile
            S_h = {}      # quad -> sim psum tile
            u_h, u2_h = {}, {}
            xt_h, qk_h, vn_h, ut_h, ot_h = {}, {}, {}, {}, {}

            def issue_x(g):
                xs = stagep.tile([P, group, DIM], BF16, tag="xs")
                xs_h[g] = xs
                r0 = (g % n_groups) * group * P
                nc.gpsimd.dma_start(
                    out=xs[:],
                    in_=x_d[r0:r0 + group * P, :].rearrange(
                        "(p t) d -> t p d", p=group))

            issue_x(0)

            for t in range(TOT + 11):
                qX = t        # x transpose
                qA = t - 2    # qkv projections + split DMAs
                qB = t - 4    # sim
                qE = t - 5    # exp + softmax
                qT = t - 7    # u transposes
                qV = t - 8    # av
                qP = t - 10   # proj + out

                # ---- exp chain start (dep finished last iter on PE) ----
                if 0 <= qE < TOT:
                    S = S_h[qE]
                    u = usp.tile([DIM, 2 * HEADS, 64], BF16, tag="u")
                    sv = S[:].rearrange("p (a j) -> p a j", j=64)
                    nc.scalar.activation(u[:, :, 0:N], sv[:, :, 0:N],
                                         func=Exp)
                    # fold exp(bias) in on the Pool engine (SBUF-only)
                    u2a = usp.tile([DIM, 2 * HEADS, 64], BF16, tag="u2a")
                    eb = expb[:]
                    eb_b = bass.AP(
                        tensor=eb.tensor, offset=eb.offset,
                        ap=[list(eb.ap[0]), [0, 2], [64, HEADS], [1, N]])
                    nc.gpsimd.tensor_mul(
                        u2a[:, :, 0:N].rearrange("p (s h) j -> p s h j", s=2),
                        u[:, :, 0:N].rearrange("p (s h) j -> p s h j", s=2),
                        eb_b)
                    u_h[qE] = u2a

                # ---- av (v2-style layout, all operands base 0) ----
                if 0 <= qV < TOT:
                    cu = CU_h.pop(qV)
                    ut = ut_h.pop(qV)
                    vn = vn_h.pop(qV)
                    av = cu[:, 0:392].rearrange(
                        "p (g w c) -> p g w c", g=2, w=4)
                    for s in range(2):
                        for w in range(2):
                            wi = 2 * s + w
                            for h in range(HEADS):
                                nc.tensor.matmul(
                                    av[DH * (h % 2):DH * (h % 2) + DH,
                                       h // 2, wi, :],
                                    lhsT=vn[0:N, wi, DH * h:DH * h + DH],
                                    rhs=ut[0:N, HEADS * s + h,
                                           64 * w:64 * w + N])
                    ot = otsp.tile([64, 2, 4, N], BF16, tag="ot")
                    ot_h[qV] = ot
                    nc.vector.tensor_copy(ot[:], av[0:64])

                # ---- W bank + x transpose ----
                if (0 <= qX < TOT) or (0 <= qA < TOT):
                    W_h[t] = psW.tile([DIM, 512], F32, tag="w", name="wbank")

                if 0 <= qX < TOT:
                    if qX % 4 == 0 and qX // 4 + 1 < TOTG:
                        issue_x(qX // 4 + 1)
                    W = W_h[t]
                    xs = xs_h[qX // 4]
                    xtp = W[:, 392:490].bitcast(BF16)    # [128, 196]
                    for s2 in range(2):
                        nc.tensor.transpose(
                            xtp[:, P * s2:P * s2 + P],
                            xs[:, 2 * (qX % 4) + s2, :],
                            i128[0:P, 0:P])
                    if qX % 4 == 3:
                        xs_h.pop(qX // 4)
                    xt = xtsp.tile([DIM, 2 * P], BF16, tag="xt")
                    xt_h[qX] = xt
                    nc.vector.tensor_copy(xt[:], xtp[:])

                # ---- u transposes (8 per-head, out rows 0:64) ----
                if 0 <= qT < TOT:
                    cu = psCU.tile([DIM, 512], F32, tag="cu")
                    CU_h[qT] = cu
                    u2 = u2_h.pop(qT)
                    utp = cu[:].bitcast(BF16)            # [128, 1024]
                    for b in range(2 * HEADS):
                        nc.tensor.transpose(
                            utp[0:64, 116 * b:116 * b + 113],
                            u2[0:113, b, :],
                            i128[0:113, 0:113])
                    ut = utsp.tile([64, 2 * HEADS, 116], BF16, tag="ut")
                    ut_h[qT] = ut
                    nc.vector.tensor_copy(
                        ut[:, :, 0:113],
                        utp[0:64, 0:928].rearrange(
                            "p (b c) -> p b c", b=8)[:, :, 0:113])

                # ---- qkv projections + head-split DMAs ----
                if 0 <= qA < TOT:
                    W = W_h[t]
                    V = psV.tile([DIM, 512], F32, tag="v")
                    xt = xt_h.pop(qA)
                    nc.tensor.matmul(W[:, 0:196], lhsT=wqS[:], rhs=xt[:])
                    nc.tensor.matmul(W[:, 196:392], lhsT=wkS[:], rhs=xt[:])
                    vv = V[:].rearrange("p (w d) -> p w d", w=4)
                    for w in range(4):
                        nc.tensor.matmul(
                            vv[0:N, w, :],
                            lhsT=xt[:, N * w:N * w + N],
                            rhs=wvS[:])
                    qk = qksp.tile([DIM, 392], BF16, tag="qk")
                    nc.scalar.copy(qk[:], W[:, 0:392])
                    qkh = qkhp.tile([32, HEADS, 392], BF16, tag="qkh")
                    qk_h[qA] = qkh
                    for h in range(HEADS):
                        eng = nc.sync if h % 2 == 0 else nc.gpsimd
                        eng.dma_start(out=qkh[:, h, :],
                                      in_=qk[32 * h:32 * h + 32, :])
                    vn = vnsp.tile([N, 4, DIM], BF16, tag="vn")
                    vn_h[qA] = vn
                    nc.scalar.copy(vn[:], vv[0:N])

                # ---- softmax (reduce / recip DVE, norm mul Pool) ----
                if 0 <= qE < TOT:
                    u2a = u_h.pop(qE)
                    sm = srp.tile([DIM, 2 * HEADS], F32, tag="sm")
                    nc.vector.tensor_reduce(
                        sm[:], u2a[:, :, 0:N],
                        axis=mybir.AxisListType.X, op=mybir.AluOpType.add)
                    r_ = srp.tile([DIM, 2 * HEADS], F32, tag="r")
                    nc.vector.reciprocal(r_[:], sm[:])
                    u2 = u2sp.tile([DIM, 2 * HEADS, 64], BF16, tag="u2")
                    u2_h[qE] = u2
                    r_b = bass.AP(
                        tensor=r_[:].tensor, offset=r_[:].offset,
                        ap=[list(r_[:].ap[0]), list(r_[:].ap[1]), [0, N]])
                    nc.gpsimd.tensor_mul(u2[:, :, 0:N], u2a[:, :, 0:N], r_b)

                # ---- proj + out staging + group out DMA ----
                if 0 <= qP < TOT:
                    Sp = S_h.pop(qP + 4)
                    ot = ot_h.pop(qP)
                    fj = Sp[:, 0:256].rearrange("p (s d) -> p s d", s=2)
                    for s2 in range(2):
                        for hh in range(2):
                            nc.tensor.matmul(
                                fj[0:P, s2, :],
                                lhsT=ot[:, hh, 2 * s2:2 * s2 + 2, :],
                                rhs=wo2[:, hh, :],
                                start=(hh == 0), stop=(hh == 1))
                    g = qP // 4
                    if qP % 4 == 0:
                        fs = finp.tile([P, group, DIM], F32, tag="fs")
                        fs_h[g] = fs
                    fs = fs_h[g]
                    lp = 2 * (qP % 4)
                    nc.scalar.copy(fs[:, lp:lp + 2, :], fj[0:P, :, :])
                    if qP % 4 == 3:
                        r0 = (g % n_groups) * group * P
                        nc.sync.dma_start(
                            out=out_d[r0:r0 + group * P, :].rearrange(
                                "(p t) d -> t p d", p=group),
                            in_=fs_h.pop(g)[:])

                # ---- sim (standalone matmuls; bias is multiplicative) ----
                if 0 <= qB < TOT + 4:
                    S = psS.tile([DIM, 512], F32, tag="s", name="sbank")
                    S_h[qB] = S
                if 0 <= qB < TOT:
                    S = S_h[qB]
                    qkh = qk_h.pop(qB)
                    for s in range(2):
                        for w in range(2):
                            for h in range(HEADS):
                                t0 = P * s + N * w
                                c0 = 256 * s + 64 * h
                                # M=64 (not 49) so every psum row is
                                # (re)written each generation; extra rows
                                # are finite garbage nothing reads
                                nc.tensor.matmul(
                                    S[64 * w:64 * w + 64, c0:c0 + N],
                                    lhsT=qkh[:, h, t0:t0 + 64],
                                    rhs=qkh[:, h, 196 + t0:196 + t0 + N])
    nc.finalize()
    return nc


_CACHE = {}


def _get_program(n_pairs):
    if n_pairs not in _CACHE:
        _CACHE[n_pairs] = build_program(n_pairs)
    return _CACHE[n_pairs]


def _host_inputs(W_qkv, W_out, bias_table):
    W_qkv = np.asarray(W_qkv, np.float32)
    wo = np.asarray(W_out, np.float32).reshape(2, 64, DIM).transpose(1, 0, 2)
    return {
        "wq": np.ascontiguousarray((W_qkv[:, :DIM] * SCALE)).astype(BF),
        "wk": np.ascontiguousarray(W_qkv[:, DIM:2 * DIM]).astype(BF),
        "wv": np.ascontiguousarray(W_qkv[:, 2 * DIM:]).astype(BF),
        "wo": np.ascontiguousarray(wo).astype(BF),
        "expb": _build_expb(bias_table).astype(BF),
        "i128": np.eye(DIM, dtype=np.float32).astype(BF),
    }


def kernel(x, W_qkv, W_out, bias_table):
    x = np.asarray(x, np.float32)
    shp = x.shape
    xf = np.ascontiguousarray(x.reshape(-1, DIM))
    tok = xf.shape[0]
    per = tok // NCORES
    n_pairs = per // P
    assert per % P == 0
    nc = _get_program(n_pairs)
    consts = _host_inputs(W_qkv, W_out, bias_table)
    in_maps = []
    for c in range(NCORES):
        m = {"x": np.ascontiguousarray(xf[c * per:(c + 1) * per])}
        m.update(consts)
        in_maps.append(m)
    res = run_bass_kernel_spmd(nc, in_maps, list(range(NCORES)))
    outs = [res.results[c]["out"] for c in range(NCORES)]
    return np.concatenate(outs, 0).reshape(shp).astype(np.float32)


# revision 50
# speedup vs baseline: 6.7272x; 5.8410x over previous
"""Swin-style windowed attention TRN2 kernel (v3.3).

Math per window (n=49 tokens, d=128, 4 heads x 32):
  qkv = x @ W_qkv ; q *= dh**-0.5
  sim[h] = q_h @ k_h^T + bias[h] ; attn = softmax_j(sim)
  out = (attn @ v) @ W_out

Fully retimed 14-stage software pipeline over QUADs (4 windows = 2
pairs = 196 tokens): every ACT/DVE/PE instruction's dependencies are
>= 1 iteration old, so engine queues never head-of-line block; only
Pool/SP (which have slack) chase same-iteration events.

Hardware constraints discovered on the way (see microtest.py):
  - tile_position row (the stationary quadrant) cannot change between
    in-flight matmuls: all operands live at partition base 0; only
    column positions {0, 32, 64} are used (v2's proven envelope).
  - Mixed-position PSUM accumulation groups corrupt results, so the
    relative-position bias is MULTIPLICATIVE: u = exp(sim)*exp(bias),
    exp(bias) precomputed; every sim matmul is a standalone group.
  - GPSIMD cannot touch PSUM; DMA cannot touch PSUM; casting DMAs run
    only on the gpsimd queue.
Head-split of the full-width q/k projection (psum rows 32h -> base 0)
is done by 3 small SBUF->SBUF DMAs (head 0 needs no rebase).

Iteration t phases (quad = t - k):
  k=0   PE xT x2 -> W(t) cols 392:490 (bf16)
  k=1   DVE xt copy
  k=2   PE q,k M=128 x2 + v x4 -> W(t) / V(t)
  k=3   ACT qk + vn copies; SP/Pool qkh rebase DMAs
  k=5   PE sim x16 (K=32, M=64, col positions 64w) -> S
  k=6   ACT exp; Pool exp(bias) mul
  k=7   DVE reduce+recip; Pool norm mul
  k=8   PE uT x8 per-head transposes [113,64]->[64,113] -> CU
  k=9   DVE ut copy
  k=10  PE av x16 -> V(t-1) bank (WAR reuse over v)
  k=11  DVE ot copy
  k=12  PE proj x2x2 K=64 -> W(t-1) cols 0:256 (WAR reuse)
  k=13  ACT fs copy; SP group out DMA
PSUM: W2 + V2 + S2 + CU2 = 8 banks, double-buffered.
"""

import os
import numpy as np
import ml_dtypes

import concourse.bass as bass
import concourse.mybir as mybir
import concourse.tile as tile
from concourse import bacc
from concourse.bass_utils import run_bass_kernel_spmd

DIM = 128
DH = 32
HEADS = 4
WS = 7
N = 49
SCALE = DH ** -0.5
P = 98            # tokens per window pair
NCORES = 8
GROUP = 8         # pairs per input DMA group (= 4 quads)

F32 = mybir.dt.float32
BF16 = mybir.dt.bfloat16
BF = ml_dtypes.bfloat16
Exp = mybir.ActivationFunctionType.Exp


def _rel_pos_bias(bias_table):
    pos = np.arange(WS)
    gi, gj = np.meshgrid(pos, pos, indexing="ij")
    grid = np.stack([gi, gj], -1).reshape(N, 2)
    rel = grid[:, None, :] - grid[None, :, :] + (WS - 1)
    idx = rel[..., 0] * (2 * WS - 1) + rel[..., 1]          # [N, N] int
    b = np.asarray(bias_table, np.float32)[idx]             # [N, N, H]
    return np.transpose(b, (2, 0, 1))                       # [H, N, N]


def _build_expb(bias_table):
    """[128, 4*64]: exp(bias), rows i repeated at 64 offset, cols (h, jp)."""
    bh = _rel_pos_bias(bias_table)
    out = np.ones((DIM, HEADS * 64), np.float32)
    for h in range(HEADS):
        blk = np.exp(bh[h])                                  # [49, 49]
        out[:N, 64 * h:64 * h + N] = blk
        out[64:64 + N, 64 * h:64 * h + N] = blk
    return out


def build_program(n_pairs, group=GROUP, repeats=1):
    nc = bacc.Bacc("TRN2", target_bir_lowering=False)
    TOK = n_pairs * P
    n_groups = n_pairs // group
    assert n_pairs % group == 0 and group == 8
    NQ = n_pairs // 2          # quads per repeat
    TOT = NQ * repeats
    TOTG = n_groups * repeats

    x_d = nc.declare_dram_parameter("x", [TOK, DIM], F32, isOutput=False)
    wq_d = nc.declare_dram_parameter("wq", [DIM, DIM], BF16, isOutput=False)
    wk_d = nc.declare_dram_parameter("wk", [DIM, DIM], BF16, isOutput=False)
    wv_d = nc.declare_dram_parameter("wv", [DIM, DIM], BF16, isOutput=False)
    wo_d = nc.declare_dram_parameter("wo", [64, 2, DIM], BF16, isOutput=False)
    expb_d = nc.declare_dram_parameter("expb", [DIM, HEADS * 64], BF16,
                                       isOutput=False)
    i128_d = nc.declare_dram_parameter("i128", [DIM, DIM], BF16, isOutput=False)
    out_d = nc.declare_dram_parameter("out", [TOK, DIM], F32, isOutput=True)

    with tile.TileContext(nc) as tc:
        with (
            tc.tile_pool(name="const", bufs=1) as constp,
            tc.tile_pool(name="stage", bufs=3) as stagep,
            tc.tile_pool(name="xts", bufs=5) as xtsp,
            tc.tile_pool(name="qks", bufs=4) as qksp,
            tc.tile_pool(name="qkh", bufs=4) as qkhp,
            tc.tile_pool(name="vns", bufs=9) as vnsp,
            tc.tile_pool(name="us", bufs=4) as usp,
            tc.tile_pool(name="u2s", bufs=5) as u2sp,
            tc.tile_pool(name="sr", bufs=4) as srp,
            tc.tile_pool(name="uts", bufs=4) as utsp,
            tc.tile_pool(name="ots", bufs=5) as otsp,
            tc.tile_pool(name="fin", bufs=3) as finp,
            tc.tile_pool(name="psW", bufs=2, space="PSUM") as psW,
            tc.tile_pool(name="psV", bufs=2, space="PSUM") as psV,
            tc.tile_pool(name="psS", bufs=2, space="PSUM") as psS,
            tc.tile_pool(name="psCU", bufs=2, space="PSUM") as psCU,
        ):
            wqS = constp.tile([DIM, DIM], BF16)
            nc.sync.dma_start(out=wqS[:], in_=wq_d[:])
            wkS = constp.tile([DIM, DIM], BF16)
            nc.sync.dma_start(out=wkS[:], in_=wk_d[:])
            wvS = constp.tile([DIM, DIM], BF16)
            nc.sync.dma_start(out=wvS[:], in_=wv_d[:])
            wo2 = constp.tile([64, 2, DIM], BF16)
            nc.sync.dma_start(out=wo2[:], in_=wo_d[:])
            expb = constp.tile([DIM, HEADS * 64], BF16)
            nc.sync.dma_start(out=expb[:], in_=expb_d[:])
            i128 = constp.tile([DIM, DIM], BF16)
            nc.sync.dma_start(out=i128[:], in_=i128_d[:])

            # pre-zero j-pad columns of all u2 slots (keeps transposed
            # garbage rows finite; av never reads them)
            for _ in range(5):
                u2z = u2sp.tile([DIM, 2 * HEADS, 64], BF16, tag="u2")
                nc.vector.memset(u2z[:, :, N:64], 0.0)

            xs_h = {}     # group -> staging tile
            fs_h = {}     # group -> out staging tile
            W_h = {}      # iter -> W psum tile
            CU_h = {}     # quad -> CU psum tile
            S_h = {}      # quad -> sim psum tile
            u_h, u2_h, V_h = {}, {}, {}
            xt_h, qk_h, vn_h, ut_h, ot_h = {}, {}, {}, {}, {}

            def issue_x(g):
                xs = stagep.tile([P, group, DIM], BF16, tag="xs")
                xs_h[g] = xs
                r0 = (g % n_groups) * group * P
                nc.gpsimd.dma_start(
                    out=xs[:],
                    in_=x_d[r0:r0 + group * P, :].rearrange(
                        "(p t) d -> t p d", p=group))

            issue_x(0)

            for t in range(TOT + 14):
                qX = t        # x transpose
                qA = t - 2    # qkv projections
                qC = t - 3    # qk/vn copies + qkh split DMAs
                qB = t - 5    # sim
                qE = t - 6    # exp + expb mul
                qR = t - 7    # reduce/recip/norm mul
                qT = t - 8    # u transposes
                qU = t - 9    # ut copy
                qV = t - 10   # av
                qO = t - 11   # ot copy
                qP = t - 12   # proj
                qF = t - 13   # fs copy + group out DMA
                W_h.pop(t - 4, None)
                V_h.pop(t - 4, None)

                # ================= old-dep front-loaded jobs =============
                # ---- exp + expb mul (ACT then Pool) ----
                if 0 <= qE < TOT:
                    S = S_h.pop(qE)
                    u = usp.tile([DIM, 2 * HEADS, 64], BF16, tag="u")
                    sv = S[:].rearrange("p (a j) -> p a j", j=64)
                    nc.scalar.activation(u[:, :, 0:N], sv[:, :, 0:N],
                                         func=Exp)
                    u2a = usp.tile([DIM, 2 * HEADS, 64], BF16, tag="u2a")
                    eb = expb[:]
                    eb_b = bass.AP(
                        tensor=eb.tensor, offset=eb.offset,
                        ap=[list(eb.ap[0]), [0, 2], [64, HEADS], [1, N]])
                    nc.gpsimd.tensor_mul(
                        u2a[:, :, 0:N].rearrange("p (s h) j -> p s h j", s=2),
                        u[:, :, 0:N].rearrange("p (s h) j -> p s h j", s=2),
                        eb_b)
                    u_h[qE] = u2a

                # ---- qk/vn copies + qkh split DMAs (ACT + SP/Pool) ----
                if 0 <= qC < TOT:
                    Wc = W_h[t - 1]
                    Vc = V_h[t - 1]
                    vn = vnsp.tile([N, 4, DIM], BF16, tag="vn")
                    vn_h[qC] = vn
                    vvc = Vc[:].rearrange("p (w d) -> p w d", w=4)
                    nc.scalar.copy(vn[:], vvc[0:N])
                    qk = qksp.tile([DIM, 392], BF16, tag="qk")
                    nc.scalar.copy(qk[:], Wc[:, 0:392])
                    qkh = qkhp.tile([32, HEADS, 392], BF16, tag="qkh")
                    qk_h[qC] = (qk, qkh)
                    for h in range(1, HEADS):
                        eng = nc.sync if h == 2 else nc.gpsimd
                        eng.dma_start(out=qkh[:, h, :],
                                      in_=qk[32 * h:32 * h + 32, :])

                # ---- reduce / recip (DVE) + norm mul (Pool) ----
                if 0 <= qR < TOT:
                    u2a = u_h.pop(qR)
                    sm = srp.tile([DIM, 2 * HEADS], F32, tag="sm")
                    nc.vector.tensor_reduce(
                        sm[:], u2a[:, :, 0:N],
                        axis=mybir.AxisListType.X, op=mybir.AluOpType.add)
                    r_ = srp.tile([DIM, 2 * HEADS], F32, tag="r")
                    nc.vector.reciprocal(r_[:], sm[:])
                    u2 = u2sp.tile([DIM, 2 * HEADS, 64], BF16, tag="u2")
                    u2_h[qR] = u2
                    r_b = bass.AP(
                        tensor=r_[:].tensor, offset=r_[:].offset,
                        ap=[list(r_[:].ap[0]), list(r_[:].ap[1]), [0, N]])
                    nc.gpsimd.tensor_mul(u2[:, :, 0:N], u2a[:, :, 0:N], r_b)

                # ---- ut copy (DVE, reads CU(qU) written last iter) ----
                if 0 <= qU < TOT:
                    cu = CU_h.pop(qU)
                    ut = utsp.tile([64, 2 * HEADS, 116], BF16, tag="ut")
                    ut_h[qU] = ut
                    nc.vector.tensor_copy(
                        ut[:, :, 0:113],
                        cu[:].bitcast(BF16)[0:64, 0:928].rearrange(
                            "p (b c) -> p b c", b=8)[:, :, 0:113])

                # ---- ot copy (DVE, reads av region in V(t-2)) ----
                if 0 <= qO < TOT:
                    Vp = V_h[t - 2]
                    avv = Vp[:, 0:392].rearrange(
                        "p (g w c) -> p g w c", g=2, w=4)
                    ot = otsp.tile([64, 2, 4, N], BF16, tag="ot")
                    ot_h[qO] = ot
                    nc.vector.tensor_copy(ot[:], avv[0:64])

                # ---- xt copy (DVE, reads W(t-1) xT region) ----
                if 0 <= t - 1 < TOT:
                    Wx = W_h[t - 1]
                    xt = xtsp.tile([DIM, 2 * P], BF16, tag="xt")
                    xt_h[t - 1] = xt
                    nc.vector.tensor_copy(xt[:], Wx[:, 392:490].bitcast(BF16))

                # ================= PE stream =============================
                # ---- W bank + x transpose ----
                if t < TOT + 12:
                    W_h[t] = psW.tile([DIM, 512], F32, tag="w", name="wbank")

                if 0 <= qX < TOT:
                    if qX % 4 == 0 and qX // 4 + 1 < TOTG:
                        issue_x(qX // 4 + 1)
                    W = W_h[t]
                    xs = xs_h[qX // 4]
                    xtp = W[:, 392:490].bitcast(BF16)    # [128, 196]
                    for s2 in range(2):
                        nc.tensor.transpose(
                            xtp[:, P * s2:P * s2 + P],
                            xs[:, 2 * (qX % 4) + s2, :],
                            i128[0:P, 0:P])
                    if qX % 4 == 3:
                        xs_h.pop(qX // 4)

                # ---- qkv projections ----
                if t < TOT + 12:
                    V_h[t] = psV.tile([DIM, 512], F32, tag="v", name="vbank")
                if 0 <= qA < TOT:
                    W = W_h[t]
                    V = V_h[t]
                    xt = xt_h.pop(qA)
                    nc.tensor.matmul(W[:, 0:196], lhsT=wqS[:], rhs=xt[:])
                    nc.tensor.matmul(W[:, 196:392], lhsT=wkS[:], rhs=xt[:])
                    vv = V[:].rearrange("p (w d) -> p w d", w=4)
                    for w in range(4):
                        nc.tensor.matmul(
                            vv[0:N, w, :],
                            lhsT=xt[:, N * w:N * w + N],
                            rhs=wvS[:])

                # ---- sim ----
                if 0 <= qB < TOT:
                    S = psS.tile([DIM, 512], F32, tag="s", name="sbank")
                    S_h[qB] = S
                    qk0, qkh = qk_h.pop(qB)
                    for s in range(2):
                        for w in range(2):
                            for h in range(HEADS):
                                t0 = P * s + N * w
                                c0 = 256 * s + 64 * h
                                if h == 0:
                                    lh = qk0[0:32, t0:t0 + 64]
                                    rh = qk0[0:32, 196 + t0:196 + t0 + N]
                                else:
                                    lh = qkh[:, h, t0:t0 + 64]
                                    rh = qkh[:, h, 196 + t0:196 + t0 + N]
                                # M=64 so every psum row is rewritten
                                # each generation (finite garbage rows)
                                nc.tensor.matmul(
                                    S[64 * w:64 * w + 64, c0:c0 + N],
                                    lhsT=lh, rhs=rh)

                # ---- u transposes ----
                if 0 <= qT < TOT:
                    cu = psCU.tile([DIM, 512], F32, tag="cu")
                    CU_h[qT] = cu
                    u2 = u2_h.pop(qT)
                    utp = cu[:].bitcast(BF16)            # [128, 1024]
                    for b in range(2 * HEADS):
                        nc.tensor.transpose(
                            utp[0:64, 116 * b:116 * b + 113],
                            u2[0:113, b, :],
                            i128[0:113, 0:113])

                # ---- av (into V(t-1), over the v region, WAR-ordered) ----
                if 0 <= qV < TOT:
                    Va = V_h[t - 1]
                    ut = ut_h.pop(qV)
                    vn = vn_h.pop(qV)
                    av = Va[:, 0:392].rearrange(
                        "p (g w c) -> p g w c", g=2, w=4)
                    for s in range(2):
                        for w in range(2):
                            wi = 2 * s + w
                            for h in range(HEADS):
                                nc.tensor.matmul(
                                    av[DH * (h % 2):DH * (h % 2) + DH,
                                       h // 2, wi, :],
                                    lhsT=vn[0:N, wi, DH * h:DH * h + DH],
                                    rhs=ut[0:N, HEADS * s + h,
                                           64 * w:64 * w + N])

                # ---- proj (into W(t-1) cols 0:256, WAR after qk copy) ----
                if 0 <= qP < TOT:
                    Wj = W_h[t - 1]
                    ot = ot_h.pop(qP)
                    fj = Wj[:, 0:256].rearrange("p (s d) -> p s d", s=2)
                    for s2 in range(2):
                        for hh in range(2):
                            nc.tensor.matmul(
                                fj[0:P, s2, :],
                                lhsT=ot[:, hh, 2 * s2:2 * s2 + 2, :],
                                rhs=wo2[:, hh, :],
                                start=(hh == 0), stop=(hh == 1))

                # ---- fs copy (ACT, reads W(t-2) proj region) ----
                if 0 <= qF < TOT:
                    Wp = W_h[t - 2]
                    fj = Wp[:, 0:256].rearrange("p (s d) -> p s d", s=2)
                    g = qF // 4
                    if qF % 4 == 0:
                        fs = finp.tile([P, group, DIM], F32, tag="fs")
                        fs_h[g] = fs
                    fs = fs_h[g]
                    lp = 2 * (qF % 4)
                    nc.scalar.copy(fs[:, lp:lp + 2, :], fj[0:P, :, :])
                    if qF % 4 == 3:
                        r0 = (g % n_groups) * group * P
                        nc.sync.dma_start(
                            out=out_d[r0:r0 + group * P, :].rearrange(
                                "(p t) d -> t p d", p=group),
                            in_=fs_h.pop(g)[:])

    nc.finalize()
    return nc


_CACHE = {}


def _get_program(n_pairs):
    if n_pairs not in _CACHE:
        _CACHE[n_pairs] = build_program(n_pairs)
    return _CACHE[n_pairs]


def _host_inputs(W_qkv, W_out, bias_table):
    W_qkv = np.asarray(W_qkv, np.float32)
    wo = np.asarray(W_out, np.float32).reshape(2, 64, DIM).transpose(1, 0, 2)
    return {
        "wq": np.ascontiguousarray((W_qkv[:, :DIM] * SCALE)).astype(BF),
        "wk": np.ascontiguousarray(W_qkv[:, DIM:2 * DIM]).astype(BF),
        "wv": np.ascontiguousarray(W_qkv[:, 2 * DIM:]).astype(BF),
        "wo": np.ascontiguousarray(wo).astype(BF),
        "expb": _build_expb(bias_table).astype(BF),
        "i128": np.eye(DIM, dtype=np.float32).astype(BF),
    }


def kernel(x, W_qkv, W_out, bias_table):
    x = np.asarray(x, np.float32)
    shp = x.shape
    xf = np.ascontiguousarray(x.reshape(-1, DIM))
    tok = xf.shape[0]
    per = tok // NCORES
    n_pairs = per // P
    assert per % P == 0
    nc = _get_program(n_pairs)
    consts = _host_inputs(W_qkv, W_out, bias_table)
    in_maps = []
    for c in range(NCORES):
        m = {"x": np.ascontiguousarray(xf[c * per:(c + 1) * per])}
        m.update(consts)
        in_maps.append(m)
    res = run_bass_kernel_spmd(nc, in_maps, list(range(NCORES)))
    outs = [res.results[c]["out"] for c in range(NCORES)]
    return np.concatenate(outs, 0).reshape(shp).astype(np.float32)
